# revision 10
# baseline (speedup 1.0000x reference)
"""Bahdanau additive attention on 8 Trainium2 NeuronCores.

reference:
    proj_f = features @ W1 + W1_b            [B, L, U]
    proj_h = hidden @ W2 + W2_b              [B, 1, U]
    attn   = tanh(proj_f + proj_h)           [B, L, U]
    score  = attn @ V + V_b                  [B, L, 1]
    wts    = softmax(score, axis=L)          [B, L, 1]
    ctx    = sum_L wts * features            [B, D]

B=256, L=1024, D=H=U=512.  Data-parallel over B: 32 batches per core.

Per-core kernel layout (per batch b, per L-chunk c of 128):
  - features_b loaded naturally as [128(l), 8(c), 512(d)] - read once.
  - PE transposes 4 [128,128] blocks -> fT_chunk [128(d), 4x128(l)] (via PSUM).
  - 4 fp32r matmuls lhsT=fT block, rhs=W1[dblock] -> proj PSUM [128(l), 512(u)].
  - DVE adds proj_h (pre-broadcast to 128 partitions per batch by GpSimd),
    ACT tanh -> attn chunk, DVE tensor_tensor_reduce (attn*V, sum) -> score.
  - Softmax without max-subtraction (scores are O(+-5)): ACT exp (+free-dim
    accum), PE ones-matmul for the cross-partition total, DVE reciprocal,
    PE rank-1 broadcast, DVE scale -> weights chunk tiles [128, 8].
  - 8 fp32r matmuls lhsT=w chunk [128,1], rhs=features chunk -> ctx [1, 512].
"""

import sys

if "/opt/trn_rl_repo" not in sys.path:
    sys.path.insert(0, "/opt/trn_rl_repo")

import numpy as np

B, L, D, H, U = 256, 1024, 512, 512, 512
N_CORES = 8
P = 128
LC = L // P  # 8 L-chunks per batch
DC = D // P  # 4 D-blocks
HC = H // P


def build_nc(b_loc, phb_eng="sync", fr_eng="gpsimd", use_f32r=True):
    import concourse.bacc as bacc
    import concourse.bass as bass
    import concourse.tile as tile
    from concourse import mybir
    from concourse.masks import make_identity

    f32 = mybir.dt.float32
    f32r = mybir.dt.float32r if use_f32r else mybir.dt.float32
    AF = mybir.ActivationFunctionType
    ALU = mybir.AluOpType

    nc = bacc.Bacc()
    features = nc.declare_dram_parameter("features", [b_loc, L, D], f32, isOutput=False)
    hidden = nc.declare_dram_parameter("hidden", [b_loc, H], f32, isOutput=False)
    W1_w = nc.declare_dram_parameter("W1_w", [D, U], f32, isOutput=False)
    W1_b = nc.declare_dram_parameter("W1_b", [U], f32, isOutput=False)
    W2_w = nc.declare_dram_parameter("W2_w", [H, U], f32, isOutput=False)
    W2_b = nc.declare_dram_parameter("W2_b", [U], f32, isOutput=False)
    V_w = nc.declare_dram_parameter("V_w", [U, 1], f32, isOutput=False)
    V_b = nc.declare_dram_parameter("V_b", [1], f32, isOutput=False)
    context = nc.declare_dram_parameter("context", [b_loc, D], f32, isOutput=True)
    ph_dram = nc.dram_tensor("ph_scratch", [b_loc, U], f32)
    weights = nc.declare_dram_parameter("weights", [b_loc, L, 1], f32, isOutput=True)

    with tile.TileContext(nc) as tc:
        with (
            tc.tile_pool(name="consts", bufs=1) as consts,
            tc.tile_pool(name="feat", bufs=2) as feat,
            tc.tile_pool(name="ftc", bufs=3) as ftc,
            tc.tile_pool(name="pre", bufs=3) as pre,
            tc.tile_pool(name="attn", bufs=3) as attn_pool,
            tc.tile_pool(name="junk", bufs=2) as junk_pool,
            tc.tile_pool(name="small", bufs=3) as small,
            tc.tile_pool(name="phb", bufs=2) as phb_pool,
            tc.tile_pool(name="tps", bufs=2, space="PSUM") as tps,
            tc.tile_pool(name="proj", bufs=3, space="PSUM") as proj_pool,
            tc.tile_pool(name="ctxp", bufs=1, space="PSUM") as ctxp,
            tc.tile_pool(name="smp", bufs=2, space="PSUM") as smp,
        ):
            # ---- one-time setup ----
            ident = consts.tile([P, P], f32)
            make_identity(nc, ident)
            ones_row = consts.tile([1, P], f32)  # [1, 128] of 1.0
            nc.gpsimd.memset(ones_row, 1.0)
            ones_col = consts.tile([P, 1], f32)  # [128, 1] of 1.0
            nc.gpsimd.memset(ones_col, 1.0)

            w1_raw = consts.tile([P, DC, U], f32)
            nc.sync.dma_start(out=w1_raw, in_=W1_w.rearrange("(j p) u -> p j u", p=P))
            w1_sb = consts.tile([P, DC, U], f32r)
            nc.vector.tensor_copy(w1_sb, w1_raw)
            w2_raw = consts.tile([P, HC, U], f32)
            nc.sync.dma_start(out=w2_raw, in_=W2_w.rearrange("(j p) u -> p j u", p=P))
            w2_sb = consts.tile([P, HC, U], f32r)
            nc.vector.tensor_copy(w2_sb, w2_raw)
            w1b_raw = consts.tile([1, U], f32)
            nc.sync.dma_start(out=w1b_raw, in_=W1_b.rearrange("(o u) -> o u", o=1))
            w1b_sb = consts.tile([1, U], f32r)
            nc.vector.tensor_copy(w1b_sb, w1b_raw)
            w2b_raw = consts.tile([1, U], f32)
            nc.sync.dma_start(out=w2b_raw, in_=W2_b.rearrange("(o u) -> o u", o=1))
            w2b_sb = consts.tile([1, U], f32r)
            nc.vector.tensor_copy(w2b_sb, w2b_raw)
            ones_row_r = consts.tile([1, P], f32r)
            nc.vector.tensor_copy(ones_row_r, ones_row)
            vt_sb = consts.tile([1, U], f32)
            nc.sync.dma_start(out=vt_sb, in_=V_w.rearrange("u o -> o u"))
            vb_sb = consts.tile([1, 1], f32)
            nc.sync.dma_start(out=vb_sb, in_=V_b.rearrange("(o u) -> o u", o=1))

            v_bc = consts.tile([P, U], f32)  # V broadcast to all partitions
            nc.gpsimd.partition_broadcast(v_bc, vt_sb)
            vb_bc = consts.tile([P, 1], f32)  # V_b broadcast
            nc.gpsimd.partition_broadcast(vb_bc, vb_sb)

            hid_sb = consts.tile([b_loc, H], f32)
            nc.sync.dma_start(out=hid_sb, in_=hidden[:, :])

            # proj_h = hidden @ W2 + W2_b + W1_b  -> ph_sb [b_loc, U]
            # (W1_b folded in so the per-chunk bias add covers both biases)
            ht_ps = tps.tile([P, P], f32, tag="t_ps")
            for j in range(HC):
                nc.tensor.transpose(
                    ht_ps[:, j * b_loc : (j + 1) * b_loc],
                    hid_sb[:, j * P : (j + 1) * P],
                    ident[:b_loc, :b_loc],
                )
            ht_sb = consts.tile([P, HC * b_loc], f32r)
            nc.scalar.copy(ht_sb, ht_ps[:, : HC * b_loc])
            ph_ps = proj_pool.tile([b_loc, U], f32, tag="proj_ps")
            for j in range(HC):
                nc.tensor.matmul(
                    ph_ps,
                    ht_sb[:, j * b_loc : (j + 1) * b_loc],
                    w2_sb[:, j, :],
                    start=(j == 0),
                    stop=False,
                )
            nc.tensor.matmul(
                ph_ps,
                ones_row_r[:, :b_loc],
                w2b_sb,
                start=False,
                stop=False,
            )
            nc.tensor.matmul(
                ph_ps,
                ones_row_r[:, :b_loc],
                w1b_sb,
                start=False,
                stop=True,
            )
            ph_sb = consts.tile([b_loc, U], f32)
            nc.scalar.copy(ph_sb, ph_ps)
            nc.sync.dma_start(out=ph_dram[:, :], in_=ph_sb)

            # ---- per-batch pipeline ----
            for b in range(b_loc):
                f_sb = feat.tile([P, LC, D], f32)
                nc.sync.dma_start(
                    out=f_sb, in_=features[b].rearrange("(c p) d -> p c d", p=P)
                )

                f_r = feat.tile([P, LC, D], f32r, tag="f_r")
                getattr(nc, fr_eng).tensor_copy(f_r, f_sb)

                # proj_h row b replicated to 128 partitions via DMA DRE
                phb = phb_pool.tile([P, U], f32)
                ph_row = ph_dram[b : b + 1, :]
                ph_rep = bass.AP(
                    tensor=ph_row.tensor, offset=ph_row.offset, ap=[[0, P], [1, U]]
                )
                getattr(nc, phb_eng).dma_start(out=phb, in_=ph_rep)

                score_t = small.tile([P, LC], f32)

                for c in range(LC):
                    # transpose features chunk: 4 blocks [128(l),128(d)] -> [128(d),128(l)]
                    t_ps = tps.tile([P, D], f32)
                    for j in range(DC):
                        nc.tensor.transpose(
                            t_ps[:, j * P : (j + 1) * P],
                            f_sb[:, c, j * P : (j + 1) * P],
                            ident,
                        )
                    ft_sb = ftc.tile([P, D], f32r)
                    nc.scalar.copy(ft_sb, t_ps)

                    proj_ps = proj_pool.tile([P, U], f32)
                    for j in range(DC):
                        nc.tensor.matmul(
                            proj_ps,
                            ft_sb[:, j * P : (j + 1) * P],
                            w1_sb[:, j, :],
                            start=(j == 0),
                            stop=(j == DC - 1),
                        )
                    # attn_pre = proj + proj_h (DVE), attn = tanh (ACT)
                    pre_sb = pre.tile([P, U], f32)
                    nc.vector.tensor_tensor(
                        out=pre_sb, in0=proj_ps, in1=phb, op=ALU.add
                    )
                    attn_sb = attn_pool.tile([P, U], f32)
                    nc.scalar.activation(attn_sb, pre_sb, AF.Tanh)
                    # score chunk = sum_u attn*V  (V_b dropped: softmax is
                    # shift-invariant so it cancels exactly)
                    junk = junk_pool.tile([P, U], f32)
                    nc.vector.tensor_tensor(out=junk, in0=attn_sb, in1=v_bc, op=ALU.mult)
                    nc.vector.tensor_reduce(
                        out=score_t[:, c : c + 1],
                        in_=junk,
                        axis=mybir.AxisListType.X,
                        op=ALU.add,
                    )

                # softmax over all 1024 scores (no max subtraction; |score|<~6)
                e_t = small.tile([P, LC], f32)
                esum = small.tile([P, 1], f32)
                nc.scalar.activation(e_t, score_t, AF.Exp, accum_out=esum)
                s_ps = smp.tile([P, 1], f32, tag="smp")
                nc.tensor.matmul(s_ps[:1, :], ones_col, esum, start=True, stop=True)
                s_sb = small.tile([1, 1], f32, tag="s_sb")
                nc.scalar.copy(s_sb, s_ps[:1, :])
                r_sb = small.tile([1, 1], f32, tag="r_sb")
                nc.vector.reciprocal(r_sb, s_sb)
                rb_ps = smp.tile([P, 1], f32, tag="smp")
                nc.tensor.matmul(rb_ps, ones_row, r_sb, start=True, stop=True)
                rb_sb = small.tile([P, 1], f32, tag="rb_sb")
                nc.scalar.copy(rb_sb, rb_ps)
                w_t = small.tile([P, LC], f32)
                nc.vector.tensor_scalar_mul(w_t, e_t, rb_sb)
                w_r = small.tile([P, LC], f32r, tag="w_r")
                nc.vector.tensor_copy(w_r, w_t)

                # context = sum_c w_chunk.T @ f_chunk  -> [1, D]
                ctx_ps = ctxp.tile([1, D], f32)
                for c in range(LC):
                    nc.tensor.matmul(
                        ctx_ps,
                        w_r[:, c : c + 1],
                        f_r[:, c, :],
                        start=(c == 0),
                        stop=(c == LC - 1),
                    )
                ctx_sb = small.tile([1, D], f32, tag="ctx_sb")
                nc.scalar.copy(ctx_sb, ctx_ps)
                nc.sync.dma_start(out=context[b : b + 1, :], in_=ctx_sb)
                for c in range(LC):
                    nc.sync.dma_start(
                        out=weights[b, c * P : (c + 1) * P, :],
                        in_=w_t[:, c : c + 1],
                    )

    nc.finalize()
    return nc


_CACHE = {}


def _get_nc(b_loc):
    if b_loc not in _CACHE:
        _CACHE[b_loc] = build_nc(b_loc)
    return _CACHE[b_loc]


def kernel(features, hidden, W1_w, W1_b, W2_w, W2_b, V_w, V_b):
    from concourse.bass_utils import run_bass_kernel_spmd

    features = np.ascontiguousarray(features, dtype=np.float32)
    hidden = np.ascontiguousarray(hidden, dtype=np.float32)
    b_loc = B // N_CORES
    nc = _get_nc(b_loc)
    shared = {
        "W1_w": np.ascontiguousarray(W1_w, dtype=np.float32),
        "W1_b": np.ascontiguousarray(W1_b, dtype=np.float32),
        "W2_w": np.ascontiguousarray(W2_w, dtype=np.float32),
        "W2_b": np.ascontiguousarray(W2_b, dtype=np.float32),
        "V_w": np.ascontiguousarray(V_w, dtype=np.float32),
        "V_b": np.ascontiguousarray(V_b, dtype=np.float32),
    }
    in_maps = [
        {
            "features": features[i * b_loc : (i + 1) * b_loc],
            "hidden": hidden[i * b_loc : (i + 1) * b_loc],
            **shared,
        }
        for i in range(N_CORES)
    ]
    res = run_bass_kernel_spmd(nc, in_maps, list(range(N_CORES)))
    ctx = np.concatenate([r["context"] for r in res.results], axis=0)
    wts = np.concatenate([r["weights"] for r in res.results], axis=0)
    return ctx, wts


# revision 20
# speedup vs baseline: 101.8069x; 101.8069x over previous
"""Bahdanau additive attention on 8 Trainium2 NeuronCores.

reference:
    proj_f = features @ W1 + W1_b            [B, L, U]
    proj_h = hidden @ W2 + W2_b              [B, 1, U]
    attn   = tanh(proj_f + proj_h)           [B, L, U]
    score  = attn @ V + V_b                  [B, L, 1]
    wts    = softmax(score, axis=L)          [B, L, 1]
    ctx    = sum_L wts * features            [B, D]

B=256, L=1024, D=H=U=512.  Data-parallel over B: 32 batches per core.

Per-core kernel layout (per batch b, per L-chunk c of 128):
  - features_b loaded naturally as [128(l), 8(c), 512(d)] - read once.
  - PE transposes 4 [128,128] blocks -> fT_chunk [128(d), 4x128(l)] (via PSUM).
  - 4 fp32r matmuls lhsT=fT block, rhs=W1[dblock] -> proj PSUM [128(l), 512(u)].
  - DVE adds proj_h (pre-broadcast to 128 partitions per batch by GpSimd),
    ACT tanh -> attn chunk, DVE tensor_tensor_reduce (attn*V, sum) -> score.
  - Softmax without max-subtraction (scores are O(+-5)): ACT exp (+free-dim
    accum), PE ones-matmul for the cross-partition total, DVE reciprocal,
    PE rank-1 broadcast, DVE scale -> weights chunk tiles [128, 8].
  - 8 fp32r matmuls lhsT=w chunk [128,1], rhs=features chunk -> ctx [1, 512].
"""

import sys

if "/opt/trn_rl_repo" not in sys.path:
    sys.path.insert(0, "/opt/trn_rl_repo")

import numpy as np

B, L, D, H, U = 256, 1024, 512, 512, 512
N_CORES = 8
P = 128
LC = L // P  # 8 L-chunks per batch
DC = D // P  # 4 D-blocks
HC = H // P


def build_nc(b_loc, phb_eng="sync", fr_eng="gpsimd", use_f32r=True, bufs=None, repeat=1):
    import concourse.bacc as bacc
    import concourse.bass as bass
    import concourse.tile as tile
    from concourse import mybir
    from concourse.masks import make_identity

    f32 = mybir.dt.float32
    f32r = mybir.dt.float32r if use_f32r else mybir.dt.float32
    AF = mybir.ActivationFunctionType
    ALU = mybir.AluOpType

    nc = bacc.Bacc()
    features = nc.declare_dram_parameter("features", [b_loc, L, D], f32, isOutput=False)
    hidden = nc.declare_dram_parameter("hidden", [b_loc, H], f32, isOutput=False)
    W1_w = nc.declare_dram_parameter("W1_w", [D, U], f32, isOutput=False)
    W1_b = nc.declare_dram_parameter("W1_b", [U], f32, isOutput=False)
    W2_w = nc.declare_dram_parameter("W2_w", [H, U], f32, isOutput=False)
    W2_b = nc.declare_dram_parameter("W2_b", [U], f32, isOutput=False)
    V_w = nc.declare_dram_parameter("V_w", [U, 1], f32, isOutput=False)
    V_b = nc.declare_dram_parameter("V_b", [1], f32, isOutput=False)
    context = nc.declare_dram_parameter("context", [b_loc, D], f32, isOutput=True)
    ph_dram = nc.dram_tensor("ph_scratch", [b_loc, U], f32)
    weights = nc.declare_dram_parameter("weights", [b_loc, L, 1], f32, isOutput=True)

    _b = dict(feat=2, ftc=3, pre=3, attn=3, junk=2, small=3, phb=2,
              tps=3, proj=2, ctxp=1, smc=2)
    _b.update(bufs or {})
    bufs = _b
    with tile.TileContext(nc) as tc:
        with (
            tc.tile_pool(name="consts", bufs=1) as consts,
            tc.tile_pool(name="feat", bufs=bufs["feat"]) as feat,
            tc.tile_pool(name="ftc", bufs=bufs["ftc"]) as ftc,
            tc.tile_pool(name="pre", bufs=bufs["pre"]) as pre,
            tc.tile_pool(name="attn", bufs=bufs["attn"]) as attn_pool,
            tc.tile_pool(name="junk", bufs=bufs["junk"]) as junk_pool,
            tc.tile_pool(name="small", bufs=bufs["small"]) as small,
            tc.tile_pool(name="phb", bufs=bufs["phb"]) as phb_pool,
            tc.tile_pool(name="tps", bufs=bufs["tps"], space="PSUM") as tps,
            tc.tile_pool(name="proj", bufs=bufs["proj"], space="PSUM") as proj_pool,
            tc.tile_pool(name="ctxp", bufs=bufs["ctxp"], space="PSUM") as ctxp,
            tc.tile_pool(name="smc", bufs=bufs["smc"], space="PSUM") as smc,
        ):
            # ---- one-time setup ----
            ident_r = consts.tile([P, P], f32r)
            ones_row = consts.tile([1, P], f32)  # [1, 128] of 1.0
            nc.gpsimd.memset(ones_row, 1.0)
            ones_col = consts.tile([P, 1], f32)  # [128, 1] of 1.0
            nc.gpsimd.memset(ones_col, 1.0)
            w1_sb = consts.tile([P, DC, U], f32r)
            w2_sb = consts.tile([P, HC, U], f32r)
            v_bc = consts.tile([P, U], f32)  # V broadcast to all partitions
            ph_sb = consts.tile([b_loc, U], f32)

            with tc.tile_pool(name="setup", bufs=1) as setup:
                ident = setup.tile([P, P], f32)
                make_identity(nc, ident)
                nc.vector.tensor_copy(ident_r, ident)
                w1_raw = setup.tile([P, DC, U], f32)
                nc.sync.dma_start(out=w1_raw, in_=W1_w.rearrange("(j p) u -> p j u", p=P))
                nc.vector.tensor_copy(w1_sb, w1_raw)
                w2_raw = setup.tile([P, HC, U], f32)
                nc.sync.dma_start(out=w2_raw, in_=W2_w.rearrange("(j p) u -> p j u", p=P))
                nc.vector.tensor_copy(w2_sb, w2_raw)
                w1b_raw = setup.tile([1, U], f32)
                nc.sync.dma_start(out=w1b_raw, in_=W1_b.rearrange("(o u) -> o u", o=1))
                w1b_sb = setup.tile([1, U], f32r)
                nc.vector.tensor_copy(w1b_sb, w1b_raw)
                w2b_raw = setup.tile([1, U], f32)
                nc.sync.dma_start(out=w2b_raw, in_=W2_b.rearrange("(o u) -> o u", o=1))
                w2b_sb = setup.tile([1, U], f32r)
                nc.vector.tensor_copy(w2b_sb, w2b_raw)
                ones_row_r = setup.tile([1, P], f32r)
                nc.vector.tensor_copy(ones_row_r, ones_row)
                vt_sb = setup.tile([1, U], f32)
                nc.sync.dma_start(out=vt_sb, in_=V_w.rearrange("u o -> o u"))
                nc.gpsimd.partition_broadcast(v_bc, vt_sb)

                hid_sb = setup.tile([b_loc, H], f32)
                nc.sync.dma_start(out=hid_sb, in_=hidden[:, :])

                # proj_h = hidden @ W2 + W2_b + W1_b  -> ph_sb [b_loc, U]
                # (W1_b folded in so the per-chunk bias add covers both biases)
                ht_ps = tps.tile([P, P], f32, tag="t_ps")
                for j in range(HC):
                    nc.tensor.transpose(
                        ht_ps[:, j * b_loc : (j + 1) * b_loc],
                        hid_sb[:, j * P : (j + 1) * P],
                        ident[:b_loc, :b_loc],
                    )
                ht_sb = setup.tile([P, HC * b_loc], f32r)
                nc.scalar.copy(ht_sb, ht_ps[:, : HC * b_loc])
                ph_ps = proj_pool.tile([b_loc, U], f32, tag="proj_ps")
                for j in range(HC):
                    nc.tensor.matmul(
                        ph_ps,
                        ht_sb[:, j * b_loc : (j + 1) * b_loc],
                        w2_sb[:, j, :],
                        start=(j == 0),
                        stop=False,
                    )
                nc.tensor.matmul(
                    ph_ps, ones_row_r[:, :b_loc], w2b_sb, start=False, stop=False
                )
                nc.tensor.matmul(
                    ph_ps, ones_row_r[:, :b_loc], w1b_sb, start=False, stop=True
                )
                nc.scalar.copy(ph_sb, ph_ps)
                nc.sync.dma_start(out=ph_dram[:, :], in_=ph_sb)

            # ---- per-batch pipeline ----
            for b in [bb for _ in range(repeat) for bb in range(b_loc)]:
                f_sb = feat.tile([P, LC, D], f32)
                f_dram = features[b].rearrange("(c p) d -> p c d", p=P)
                for c in range(LC):
                    nc.sync.dma_start(out=f_sb[:, c, :], in_=f_dram[:, c, :])

                f_r = feat.tile([P, LC, D], f32r, tag="f_r")
                for c in range(LC):
                    getattr(nc, fr_eng).tensor_copy(f_r[:, c, :], f_sb[:, c, :])

                # proj_h row b replicated to 128 partitions via DMA DRE
                phb = phb_pool.tile([P, U], f32)
                ph_row = ph_dram[b : b + 1, :]
                ph_rep = bass.AP(
                    tensor=ph_row.tensor, offset=ph_row.offset, ap=[[0, P], [1, U]]
                )
                getattr(nc, phb_eng).dma_start(out=phb, in_=ph_rep)

                score_t = small.tile([P, LC], f32)

                for c in range(LC):
                    # transpose features chunk: 4 blocks [128(l),128(d)] -> [128(d),128(l)]
                    t_ps = tps.tile([P, D], f32r, tag="t_ps")
                    for j in range(DC):
                        nc.tensor.transpose(
                            t_ps[:, j * P : (j + 1) * P],
                            f_r[:, c, j * P : (j + 1) * P],
                            ident_r,
                        )
                    ft_sb = ftc.tile([P, D], f32r)
                    nc.scalar.copy(ft_sb, t_ps)

                    proj_ps = proj_pool.tile([P, U], f32)
                    for j in range(DC):
                        nc.tensor.matmul(
                            proj_ps,
                            ft_sb[:, j * P : (j + 1) * P],
                            w1_sb[:, j, :],
                            start=(j == 0),
                            stop=(j == DC - 1),
                        )
                    # attn_pre = proj + proj_h (DVE), attn = tanh (ACT)
                    pre_sb = pre.tile([P, U], f32)
                    nc.vector.tensor_tensor(
                        out=pre_sb, in0=proj_ps, in1=phb, op=ALU.add
                    )
                    attn_sb = attn_pool.tile([P, U], f32)
                    nc.scalar.activation(attn_sb, pre_sb, AF.Tanh)
                    # score chunk = sum_u attn*V in one fused DVE pass
                    # (V_b dropped: softmax is shift-invariant, it cancels)
                    junk = junk_pool.tile([P, U], f32)
                    nc.vector.scalar_tensor_tensor(
                        out=junk,
                        in0=attn_sb,
                        scalar=1.0,
                        in1=v_bc,
                        op0=ALU.mult,
                        op1=ALU.mult,
                        accum_out=score_t[:, c : c + 1],
                    )

                # softmax over all 1024 scores (no max subtraction; |score|<~6)
                e_t = small.tile([P, LC], f32)
                esum = small.tile([P, 1], f32)
                nc.scalar.activation(e_t, score_t, AF.Exp, accum_out=esum)
                # context uses UNNORMALIZED e weights (scaled by 1/S at the
                # end) so the ctx matmuls depend only on exp, not the sum.
                e_r = small.tile([P, LC], f32r, tag="e_r")
                nc.vector.tensor_copy(e_r, e_t)
                ctx_ps = ctxp.tile([1, D], f32)
                for c in range(LC):
                    nc.tensor.matmul(
                        ctx_ps,
                        e_r[:, c : c + 1],
                        f_r[:, c, :],
                        start=(c == 0),
                        stop=(c == LC - 1),
                    )
                # total S = ones.T @ esum; r = 1/S
                s_ps = smc.tile([P, 1], f32, tag="smc")
                nc.tensor.matmul(s_ps[:1, :], ones_col, esum, start=True, stop=True)
                r_sb = small.tile([1, 1], f32, tag="r_sb")
                nc.vector.reciprocal(r_sb, s_ps[:1, :])
                # weights = e * r (broadcast r to [128,1] via rank-1 matmul)
                rb_ps = smc.tile([P, 1], f32, tag="smc")
                nc.tensor.matmul(rb_ps, ones_row, r_sb, start=True, stop=True)
                w_t = small.tile([P, LC], f32)
                nc.vector.tensor_scalar_mul(w_t, e_t, rb_ps)
                # context = ctx_raw * r
                ctx_sb = small.tile([1, D], f32, tag="ctx_sb")
                nc.vector.tensor_scalar_mul(ctx_sb, ctx_ps, r_sb)
                nc.sync.dma_start(out=context[b : b + 1, :], in_=ctx_sb)
                for c in range(LC):
                    nc.sync.dma_start(
                        out=weights[b, c * P : (c + 1) * P, :],
                        in_=w_t[:, c : c + 1],
                    )

    nc.finalize()
    return nc


_CACHE = {}


def _get_nc(b_loc):
    if b_loc not in _CACHE:
        _CACHE[b_loc] = build_nc(b_loc)
    return _CACHE[b_loc]


def kernel(features, hidden, W1_w, W1_b, W2_w, W2_b, V_w, V_b):
    from concourse.bass_utils import run_bass_kernel_spmd

    features = np.ascontiguousarray(features, dtype=np.float32)
    hidden = np.ascontiguousarray(hidden, dtype=np.float32)
    b_loc = B // N_CORES
    nc = _get_nc(b_loc)
    shared = {
        "W1_w": np.ascontiguousarray(W1_w, dtype=np.float32),
        "W1_b": np.ascontiguousarray(W1_b, dtype=np.float32),
        "W2_w": np.ascontiguousarray(W2_w, dtype=np.float32),
        "W2_b": np.ascontiguousarray(W2_b, dtype=np.float32),
        "V_w": np.ascontiguousarray(V_w, dtype=np.float32),
        "V_b": np.ascontiguousarray(V_b, dtype=np.float32),
    }
    in_maps = [
        {
            "features": features[i * b_loc : (i + 1) * b_loc],
            "hidden": hidden[i * b_loc : (i + 1) * b_loc],
            **shared,
        }
        for i in range(N_CORES)
    ]
    res = run_bass_kernel_spmd(nc, in_maps, list(range(N_CORES)))
    ctx = np.concatenate([r["context"] for r in res.results], axis=0)
    wts = np.concatenate([r["weights"] for r in res.results], axis=0)
    return ctx, wts


# revision 22
# speedup vs baseline: 127.3632x; 1.2510x over previous
"""Bahdanau additive attention on 8 Trainium2 NeuronCores.

reference:
    proj_f = features @ W1 + W1_b            [B, L, U]
    proj_h = hidden @ W2 + W2_b              [B, 1, U]
    attn   = tanh(proj_f + proj_h)           [B, L, U]
    score  = attn @ V + V_b                  [B, L, 1]
    wts    = softmax(score, axis=L)          [B, L, 1]
    ctx    = sum_L wts * features            [B, D]

B=256, L=1024, D=H=U=512.  Data-parallel over B: 32 batches per core.

Per-core kernel layout (per batch b, per L-chunk c of 128):
  - features_b loaded naturally as [128(l), 8(c), 512(d)] - read once.
  - PE transposes 4 [128,128] blocks -> fT_chunk [128(d), 4x128(l)] (via PSUM).
  - 4 fp32r matmuls lhsT=fT block, rhs=W1[dblock] -> proj PSUM [128(l), 512(u)].
  - DVE adds proj_h (pre-broadcast to 128 partitions per batch by GpSimd),
    ACT tanh -> attn chunk, DVE tensor_tensor_reduce (attn*V, sum) -> score.
  - Softmax without max-subtraction (scores are O(+-5)): ACT exp (+free-dim
    accum), PE ones-matmul for the cross-partition total, DVE reciprocal,
    PE rank-1 broadcast, DVE scale -> weights chunk tiles [128, 8].
  - 8 fp32r matmuls lhsT=w chunk [128,1], rhs=features chunk -> ctx [1, 512].
"""

import sys

if "/opt/trn_rl_repo" not in sys.path:
    sys.path.insert(0, "/opt/trn_rl_repo")

import numpy as np

B, L, D, H, U = 256, 1024, 512, 512, 512
N_CORES = 8
P = 128
LC = L // P  # 8 L-chunks per batch
DC = D // P  # 4 D-blocks
HC = H // P


def build_nc(b_loc, phb_eng="sync", fr_eng="gpsimd", use_f32r=True, bufs=None, repeat=1, bias_pe=True):
    import concourse.bacc as bacc
    import concourse.bass as bass
    import concourse.tile as tile
    from concourse import mybir
    from concourse.masks import make_identity

    f32 = mybir.dt.float32
    f32r = mybir.dt.float32r if use_f32r else mybir.dt.float32
    AF = mybir.ActivationFunctionType
    ALU = mybir.AluOpType

    nc = bacc.Bacc()
    features = nc.declare_dram_parameter("features", [b_loc, L, D], f32, isOutput=False)
    hidden = nc.declare_dram_parameter("hidden", [b_loc, H], f32, isOutput=False)
    W1_w = nc.declare_dram_parameter("W1_w", [D, U], f32, isOutput=False)
    W1_b = nc.declare_dram_parameter("W1_b", [U], f32, isOutput=False)
    W2_w = nc.declare_dram_parameter("W2_w", [H, U], f32, isOutput=False)
    W2_b = nc.declare_dram_parameter("W2_b", [U], f32, isOutput=False)
    V_w = nc.declare_dram_parameter("V_w", [U, 1], f32, isOutput=False)
    V_b = nc.declare_dram_parameter("V_b", [1], f32, isOutput=False)
    context = nc.declare_dram_parameter("context", [b_loc, D], f32, isOutput=True)
    ph_dram = nc.dram_tensor("ph_scratch", [b_loc, U], f32)
    weights = nc.declare_dram_parameter("weights", [b_loc, L, 1], f32, isOutput=True)

    _b = dict(feat=2, ftc=3, pre=3, attn=3, junk=2, small=3, phb=2,
              tps=3, proj=2, ctxp=1, smc=2)
    _b.update(bufs or {})
    bufs = _b
    with tile.TileContext(nc) as tc:
        with (
            tc.tile_pool(name="consts", bufs=1) as consts,
            tc.tile_pool(name="feat", bufs=bufs["feat"]) as feat,
            tc.tile_pool(name="ftc", bufs=bufs["ftc"]) as ftc,
            tc.tile_pool(name="pre", bufs=bufs["pre"]) as pre,
            tc.tile_pool(name="attn", bufs=bufs["attn"]) as attn_pool,
            tc.tile_pool(name="junk", bufs=bufs["junk"]) as junk_pool,
            tc.tile_pool(name="small", bufs=bufs["small"]) as small,
            tc.tile_pool(name="phb", bufs=bufs["phb"]) as phb_pool,
            tc.tile_pool(name="tps", bufs=bufs["tps"], space="PSUM") as tps,
            tc.tile_pool(name="proj", bufs=bufs["proj"], space="PSUM") as proj_pool,
            tc.tile_pool(name="ctxp", bufs=bufs["ctxp"], space="PSUM") as ctxp,
            tc.tile_pool(name="smc", bufs=bufs["smc"], space="PSUM") as smc,
        ):
            # ---- one-time setup ----
            ident_r = consts.tile([P, P], f32r)
            ones_row = consts.tile([1, P], f32)  # [1, 128] of 1.0
            nc.gpsimd.memset(ones_row, 1.0)
            ones_col = consts.tile([P, 1], f32)  # [128, 1] of 1.0
            nc.gpsimd.memset(ones_col, 1.0)
            w1_sb = consts.tile([P, DC, U], f32r)
            w2_sb = consts.tile([P, HC, U], f32r)
            v_bc = consts.tile([P, U], f32)  # V broadcast to all partitions
            ph_sb = consts.tile([b_loc, U], f32)
            ones_row_r = consts.tile([1, P], f32r)

            with tc.tile_pool(name="setup", bufs=1) as setup:
                ident = setup.tile([P, P], f32)
                make_identity(nc, ident)
                nc.vector.tensor_copy(ident_r, ident)
                w1_raw = setup.tile([P, DC, U], f32)
                nc.sync.dma_start(out=w1_raw, in_=W1_w.rearrange("(j p) u -> p j u", p=P))
                nc.vector.tensor_copy(w1_sb, w1_raw)
                w2_raw = setup.tile([P, HC, U], f32)
                nc.sync.dma_start(out=w2_raw, in_=W2_w.rearrange("(j p) u -> p j u", p=P))
                nc.vector.tensor_copy(w2_sb, w2_raw)
                w1b_raw = setup.tile([1, U], f32)
                nc.sync.dma_start(out=w1b_raw, in_=W1_b.rearrange("(o u) -> o u", o=1))
                w1b_sb = setup.tile([1, U], f32r)
                nc.vector.tensor_copy(w1b_sb, w1b_raw)
                w2b_raw = setup.tile([1, U], f32)
                nc.sync.dma_start(out=w2b_raw, in_=W2_b.rearrange("(o u) -> o u", o=1))
                w2b_sb = setup.tile([1, U], f32r)
                nc.vector.tensor_copy(w2b_sb, w2b_raw)
                nc.vector.tensor_copy(ones_row_r, ones_row)
                vt_sb = setup.tile([1, U], f32)
                nc.sync.dma_start(out=vt_sb, in_=V_w.rearrange("u o -> o u"))
                nc.gpsimd.partition_broadcast(v_bc, vt_sb)

                hid_sb = setup.tile([b_loc, H], f32)
                nc.sync.dma_start(out=hid_sb, in_=hidden[:, :])

                # proj_h = hidden @ W2 + W2_b + W1_b  -> ph_sb [b_loc, U]
                # (W1_b folded in so the per-chunk bias add covers both biases)
                ht_ps = tps.tile([P, P], f32, tag="t_ps")
                for j in range(HC):
                    nc.tensor.transpose(
                        ht_ps[:, j * b_loc : (j + 1) * b_loc],
                        hid_sb[:, j * P : (j + 1) * P],
                        ident[:b_loc, :b_loc],
                    )
                ht_sb = setup.tile([P, HC * b_loc], f32r)
                nc.scalar.copy(ht_sb, ht_ps[:, : HC * b_loc])
                ph_ps = proj_pool.tile([b_loc, U], f32, tag="proj_ps")
                for j in range(HC):
                    nc.tensor.matmul(
                        ph_ps,
                        ht_sb[:, j * b_loc : (j + 1) * b_loc],
                        w2_sb[:, j, :],
                        start=(j == 0),
                        stop=False,
                    )
                nc.tensor.matmul(
                    ph_ps, ones_row_r[:, :b_loc], w2b_sb, start=False, stop=False
                )
                nc.tensor.matmul(
                    ph_ps, ones_row_r[:, :b_loc], w1b_sb, start=False, stop=True
                )
                nc.scalar.copy(ph_sb, ph_ps)
                nc.sync.dma_start(out=ph_dram[:, :], in_=ph_sb)

            # ---- per-batch pipeline ----
            for b in [bb for _ in range(repeat) for bb in range(b_loc)]:
                f_sb = feat.tile([P, LC, D], f32)
                f_dram = features[b].rearrange("(c p) d -> p c d", p=P)
                for c in range(LC):
                    nc.sync.dma_start(out=f_sb[:, c, :], in_=f_dram[:, c, :])

                f_r = feat.tile([P, LC, D], f32r, tag="f_r")
                for c in range(LC):
                    getattr(nc, fr_eng).tensor_copy(f_r[:, c, :], f_sb[:, c, :])

                # proj_h row b -> partition 0, rounded to f32r for the
                # rank-1 bias matmul in each chunk's accumulation group
                ph_row_sb = phb_pool.tile([1, U], f32, tag="ph_row")
                nc.sync.dma_start(out=ph_row_sb, in_=ph_dram[b : b + 1, :])
                ph_row_r = phb_pool.tile([1, U], f32r, tag="ph_row_r")
                nc.vector.tensor_copy(ph_row_r, ph_row_sb)

                score_t = small.tile([P, LC], f32)

                for c in range(LC):
                    # transpose features chunk: 4 blocks [128(l),128(d)] -> [128(d),128(l)]
                    t_ps = tps.tile([P, D], f32r, tag="t_ps")
                    for j in range(DC):
                        nc.tensor.transpose(
                            t_ps[:, j * P : (j + 1) * P],
                            f_r[:, c, j * P : (j + 1) * P],
                            ident_r,
                        )
                    ft_sb = ftc.tile([P, D], f32r)
                    nc.scalar.copy(ft_sb, t_ps)

                    proj_ps = proj_pool.tile([P, U], f32)
                    for j in range(DC):
                        nc.tensor.matmul(
                            proj_ps,
                            ft_sb[:, j * P : (j + 1) * P],
                            w1_sb[:, j, :],
                            start=(j == 0),
                            stop=False,
                        )
                    # += ones(128) (x) proj_h  rank-1, closes the group
                    nc.tensor.matmul(
                        proj_ps, ones_row_r, ph_row_r, start=False, stop=True
                    )
                    attn_sb = attn_pool.tile([P, U], f32)
                    nc.scalar.activation(attn_sb, proj_ps, AF.Tanh)
                    # score chunk = sum_u attn*V in one fused DVE pass
                    # (V_b dropped: softmax is shift-invariant, it cancels)
                    junk = junk_pool.tile([P, U], f32)
                    nc.vector.scalar_tensor_tensor(
                        out=junk,
                        in0=attn_sb,
                        scalar=1.0,
                        in1=v_bc,
                        op0=ALU.mult,
                        op1=ALU.mult,
                        accum_out=score_t[:, c : c + 1],
                    )

                # softmax over all 1024 scores (no max subtraction; |score|<~6)
                e_t = small.tile([P, LC], f32)
                esum = small.tile([P, 1], f32)
                nc.scalar.activation(e_t, score_t, AF.Exp, accum_out=esum)
                # context uses UNNORMALIZED e weights (scaled by 1/S at the
                # end) so the ctx matmuls depend only on exp, not the sum.
                e_r = small.tile([P, LC], f32r, tag="e_r")
                nc.vector.tensor_copy(e_r, e_t)
                ctx_ps = ctxp.tile([1, D], f32)
                for c in range(LC):
                    nc.tensor.matmul(
                        ctx_ps,
                        e_r[:, c : c + 1],
                        f_r[:, c, :],
                        start=(c == 0),
                        stop=(c == LC - 1),
                    )
                # total S = ones.T @ esum; r = 1/S
                s_ps = smc.tile([P, 1], f32, tag="smc")
                nc.tensor.matmul(s_ps[:1, :], ones_col, esum, start=True, stop=True)
                r_sb = small.tile([1, 1], f32, tag="r_sb")
                nc.vector.reciprocal(r_sb, s_ps[:1, :])
                # weights = e * r (broadcast r to [128,1] via rank-1 matmul)
                rb_ps = smc.tile([P, 1], f32, tag="smc")
                nc.tensor.matmul(rb_ps, ones_row, r_sb, start=True, stop=True)
                w_t = small.tile([P, LC], f32)
                nc.vector.tensor_scalar_mul(w_t, e_t, rb_ps)
                # context = ctx_raw * r
                ctx_sb = small.tile([1, D], f32, tag="ctx_sb")
                nc.vector.tensor_scalar_mul(ctx_sb, ctx_ps, r_sb)
                nc.sync.dma_start(out=context[b : b + 1, :], in_=ctx_sb)
                for c in range(LC):
                    nc.sync.dma_start(
                        out=weights[b, c * P : (c + 1) * P, :],
                        in_=w_t[:, c : c + 1],
                    )

    nc.finalize()
    return nc


_CACHE = {}


def _get_nc(b_loc):
    if b_loc not in _CACHE:
        _CACHE[b_loc] = build_nc(b_loc)
    return _CACHE[b_loc]


def kernel(features, hidden, W1_w, W1_b, W2_w, W2_b, V_w, V_b):
    from concourse.bass_utils import run_bass_kernel_spmd

    features = np.ascontiguousarray(features, dtype=np.float32)
    hidden = np.ascontiguousarray(hidden, dtype=np.float32)
    b_loc = B // N_CORES
    nc = _get_nc(b_loc)
    shared = {
        "W1_w": np.ascontiguousarray(W1_w, dtype=np.float32),
        "W1_b": np.ascontiguousarray(W1_b, dtype=np.float32),
        "W2_w": np.ascontiguousarray(W2_w, dtype=np.float32),
        "W2_b": np.ascontiguousarray(W2_b, dtype=np.float32),
        "V_w": np.ascontiguousarray(V_w, dtype=np.float32),
        "V_b": np.ascontiguousarray(V_b, dtype=np.float32),
    }
    in_maps = [
        {
            "features": features[i * b_loc : (i + 1) * b_loc],
            "hidden": hidden[i * b_loc : (i + 1) * b_loc],
            **shared,
        }
        for i in range(N_CORES)
    ]
    res = run_bass_kernel_spmd(nc, in_maps, list(range(N_CORES)))
    ctx = np.concatenate([r["context"] for r in res.results], axis=0)
    wts = np.concatenate([r["weights"] for r in res.results], axis=0)
    return ctx, wts
